# revision 2
# baseline (speedup 1.0000x reference)
"""ANFIS kernel for 8 TRN2 NeuronCores — pure batch data-parallel.

Math: out[b,o] = prod_f(x[b,f]) * w[b,o]^32 where
  w = sum_r(p_r * m_r) / sum_r(m_r),  m_r = exp(-((y-c_r)/s_r)^2),
  y = MLP(x).  exp(-z^2) is computed on the ScalarEngine as
  Derivative_Erf(scale*y + bias) (= 2/sqrt(pi) * exp(-z^2); the constant
  cancels in the normalization).  D = sum m and N = sum p*m are reduced
  over rules with fp16 TensorE matmuls (identity / diag(p) stationary),
  accumulating in f32 PSUM.
"""
import sys

if "/opt/trn_rl_repo" not in sys.path:
    sys.path.insert(0, "/opt/trn_rl_repo")

import numpy as np

import concourse.bacc as bacc
import concourse.mybir as mybir
from concourse.bass_utils import run_bass_kernel_spmd
from concourse.tile import TileContext
from concourse.mybir import AluOpType as Op

B, IN_DIM, OUT_DIM, N_RULES, H = 8192, 32, 256, 16, 256
N_CORES = 8
BL = B // N_CORES          # 1024 batch rows per core
P = 128                    # partitions
NOT = OUT_DIM // P         # 2 o-tiles
NJ = H // P                # 2 hidden j-tiles
FD = 512                   # matmul free-dim chunk (one PSUM bank)
F32 = mybir.dt.float32
F16 = mybir.dt.float16
SQUARE = None  # set in _build

_nc_cache = None


def _build():
    global _nc_cache
    if _nc_cache is not None:
        return _nc_cache
    nc = bacc.Bacc(None, target_bir_lowering=False, debug=False, num_devices=N_CORES)

    xT_d = nc.declare_dram_parameter("xT", [IN_DIM, BL], F32, isOutput=False)
    xbp_d = nc.declare_dram_parameter("xbp", [P, (BL // P) * IN_DIM], F32, isOutput=False)
    W1_d = nc.declare_dram_parameter("W1", [IN_DIM, H], F32, isOutput=False)
    W2p_d = nc.declare_dram_parameter("W2p", [P, NJ * NJ * P], F32, isOutput=False)
    W3p_d = nc.declare_dram_parameter("W3p", [P, NJ * NOT * P], F32, isOutput=False)
    b1t_d = nc.declare_dram_parameter("b1t", [P, NJ], F32, isOutput=False)
    b2t_d = nc.declare_dram_parameter("b2t", [P, NJ], F32, isOutput=False)
    scl_d = nc.declare_dram_parameter("scl", [P, NOT * N_RULES], F32, isOutput=False)
    bia_d = nc.declare_dram_parameter("bia", [P, NOT * N_RULES], F32, isOutput=False)
    dgs_d = nc.declare_dram_parameter("dgs", [P, NOT * N_RULES * P], F16, isOutput=False)
    eye16_d = nc.declare_dram_parameter("eye16", [P, P], F16, isOutput=False)
    eyef_d = nc.declare_dram_parameter("eyef", [P, P], F32, isOutput=False)
    ones1_d = nc.declare_dram_parameter("ones1", [1, P], F32, isOutput=False)
    out_d = nc.declare_dram_parameter("out", [OUT_DIM, BL], F32, isOutput=True)

    DERF = mybir.ActivationFunctionType.Derivative_Erf
    SQ = mybir.ActivationFunctionType.Square

    with TileContext(nc) as tc:
        with tc.sbuf_pool(name="sb", bufs=1) as sb:
            # ---- loads, ordered so PE-warmup + P + L1 can start early ----
            eyef = sb.tile([P, P], F32)
            nc.sync.dma_start(out=eyef[:], in_=eyef_d[:])
            ones1 = sb.tile([1, P], F32)
            nc.sync.dma_start(out=ones1[:], in_=ones1_d[:])
            xbp = sb.tile([P, (BL // P) * IN_DIM], F32)
            nc.sync.dma_start(out=xbp[:], in_=xbp_d[:])
            xT = sb.tile([IN_DIM, BL], F32)
            nc.sync.dma_start(out=xT[:], in_=xT_d[:])
            W1 = sb.tile([IN_DIM, H], F32)
            nc.sync.dma_start(out=W1[:], in_=W1_d[:])
            b1t = sb.tile([P, NJ], F32)
            nc.sync.dma_start(out=b1t[:], in_=b1t_d[:])
            W2p = sb.tile([P, NJ * NJ * P], F32)
            nc.sync.dma_start(out=W2p[:], in_=W2p_d[:])
            b2t = sb.tile([P, NJ], F32)
            nc.sync.dma_start(out=b2t[:], in_=b2t_d[:])
            W3p = sb.tile([P, NJ * NOT * P], F32)
            nc.sync.dma_start(out=W3p[:], in_=W3p_d[:])
            scl = sb.tile([P, NOT * N_RULES], F32)
            nc.sync.dma_start(out=scl[:], in_=scl_d[:])
            bia = sb.tile([P, NOT * N_RULES], F32)
            nc.sync.dma_start(out=bia[:], in_=bia_d[:])
            eye16 = sb.tile([P, P], F16)
            nc.sync.dma_start(out=eye16[:], in_=eye16_d[:])
            dgs = sb.tile([P, NOT * N_RULES * P], F16)
            for ot in range(NOT):
                half = slice(ot * N_RULES * P, (ot + 1) * N_RULES * P)
                nc.sync.dma_start(out=dgs[:, half], in_=dgs_d[:, half])

            # ---- PE warm-up + P[b] replication, while the rest of the DMAs land ----
            P_rep = sb.tile([P, BL], F32)
            with tc.psum_pool(name="ps_p", bufs=1) as ps_p:
                warm = ps_p.tile([P, P], F32, tag="warm")
                for _ in range(24):
                    nc.tensor.matmul(warm[:], eyef[:], eyef[:], start=True, stop=True)

                P_all = sb.tile([P, BL // P], F32)
                nc.vector.tensor_reduce(
                    P_all[:],
                    xbp[:].rearrange("p (t f) -> p t f", f=IN_DIM),
                    mybir.AxisListType.X, Op.mult,
                )
                pT = ps_p.tile([BL // P, P], F32, tag="pT")
                nc.tensor.transpose(pT[:], P_all[:], eyef[:])
                P_rowT = sb.tile([BL // P, P], F32)
                nc.vector.tensor_copy(P_rowT[:], pT[:])
                P_row = sb.tile([1, BL], F32)
                for t in range(BL // P):
                    nc.sync.dma_start(out=P_row[0:1, t * P:(t + 1) * P], in_=P_rowT[t:t + 1, :])
                prep = ps_p.tile([P, BL], F32, tag="prep")
                for c in range(BL // FD):
                    nc.tensor.matmul(prep[:, c * FD:(c + 1) * FD],
                                     ones1[:], P_row[0:1, c * FD:(c + 1) * FD],
                                     start=True, stop=True)
                nc.vector.tensor_copy(P_rep[:], prep[:])

            hT = []
            h2T = []
            with tc.psum_pool(name="ps_y", bufs=2) as ps_y:
                yT = []
                with tc.psum_pool(name="ps_mlp", bufs=2) as ps_mlp:
                    # ---- L1 ----
                    for j in range(NJ):
                        l1 = ps_mlp.tile([P, BL], F32, tag="mlp")
                        for c in range(BL // FD):
                            nc.tensor.matmul(
                                l1[:, c * FD:(c + 1) * FD],
                                W1[:, j * P:(j + 1) * P],
                                xT[:, c * FD:(c + 1) * FD],
                                start=True, stop=True,
                            )
                        h = sb.tile([P, BL], F32, name=f"hT{j}")
                        nc.vector.tensor_scalar(h[:], l1[:], b1t[:, j:j + 1], 0.0, Op.add, Op.max)
                        hT.append(h)
                    # ---- L2 ----
                    for j in range(NJ):
                        l2 = ps_mlp.tile([P, BL], F32, tag="mlp")
                        for c in range(BL // FD):
                            for k in range(NJ):
                                nc.tensor.matmul(
                                    l2[:, c * FD:(c + 1) * FD],
                                    W2p[:, (k * NJ + j) * P:(k * NJ + j + 1) * P],
                                    hT[k][:, c * FD:(c + 1) * FD],
                                    start=(k == 0), stop=(k == NJ - 1),
                                )
                        h = sb.tile([P, BL], F32, name=f"h2T{j}")
                        nc.vector.tensor_scalar(h[:], l2[:], b2t[:, j:j + 1], 0.0, Op.add, Op.max)
                        h2T.append(h)
                    # ---- L3 -> PSUM (b3 folded into activation bias) ----
                    for j in range(NOT):
                        l3 = ps_y.tile([P, BL], F32, tag="yt")
                        for c in range(BL // FD):
                            for k in range(NJ):
                                nc.tensor.matmul(
                                    l3[:, c * FD:(c + 1) * FD],
                                    W3p[:, (k * NOT + j) * P:(k * NOT + j + 1) * P],
                                    h2T[k][:, c * FD:(c + 1) * FD],
                                    start=(k == 0), stop=(k == NJ - 1),
                                )
                        yT.append(l3)

                # ---- memberships + D/N + w per o-tile ----
                with tc.psum_pool(name="ps_dn", bufs=1) as ps_dn:
                    for ot in range(NOT):
                        D = ps_dn.tile([P, BL], F32, tag="D", name=f"D{ot}")
                        N = ps_dn.tile([P, BL], F32, tag="N", name=f"N{ot}")
                        for r in range(N_RULES):
                            idx = ot * N_RULES + r
                            m = sb.tile([P, BL], F16, tag="m", bufs=4, name=f"m{idx}")
                            nc.scalar.activation(
                                m[:], yT[ot][:], DERF,
                                bias=bia[:, idx:idx + 1], scale=scl[:, idx:idx + 1],
                            )
                            for c in range(BL // FD):
                                cs = slice(c * FD, (c + 1) * FD)
                                nc.tensor.matmul(D[:, cs], eye16[:], m[:, cs],
                                                 start=(r == 0), stop=(r == N_RULES - 1))
                                nc.tensor.matmul(N[:, cs], dgs[:, idx * P:(idx + 1) * P], m[:, cs],
                                                 start=(r == 0), stop=(r == N_RULES - 1))
                        rD = sb.tile([P, BL], F32, tag="rD", bufs=2, name=f"rD{ot}")
                        nc.vector.reciprocal_approx_fast(rD[:], D[:])
                        w = sb.tile([P, BL], F32, tag="w", bufs=2, name=f"w{ot}")
                        nc.vector.tensor_tensor(w[:], N[:], rD[:], Op.mult)
                        if ot < NOT - 1:
                            for _ in range(5):
                                nc.vector.tensor_tensor(w[:], w[:], w[:], Op.mult)
                        else:
                            # last o-tile is the serial tail: pipeline the five
                            # squarings across DVE (low half) + ACT Square (high half)
                            h0 = slice(0, BL // 2)
                            h1 = slice(BL // 2, BL)
                            for _ in range(5):
                                nc.vector.tensor_tensor(w[:, h0], w[:, h0], w[:, h0], Op.mult)
                                nc.scalar.activation(w[:, h1], w[:, h1], SQ)
                        o = sb.tile([P, BL], F32, tag="osb", bufs=2, name=f"osb{ot}")
                        nc.vector.tensor_tensor(o[:], w[:], P_rep[:], Op.mult)
                        nc.sync.dma_start(out=out_d[ot * P:(ot + 1) * P, :], in_=o[:])

    nc.finalize()
    _nc_cache = nc
    return nc


def _prepare_in_maps(x, W1, b1, W2, b2, W3, b3, centers, widths, params):
    x = np.ascontiguousarray(x, dtype=np.float32)
    W1 = np.asarray(W1, np.float32); b1 = np.asarray(b1, np.float32)
    W2 = np.asarray(W2, np.float32); b2 = np.asarray(b2, np.float32)
    W3 = np.asarray(W3, np.float32); b3 = np.asarray(b3, np.float32)
    centers = np.asarray(centers, np.float32)
    widths = np.asarray(widths, np.float32)
    params = np.asarray(params, np.float32)

    def pack_w(W, nj_out):
        blocks = []
        for k in range(W.shape[0] // P):
            for j in range(nj_out):
                blocks.append(W[k * P:(k + 1) * P, j * P:(j + 1) * P])
        return np.ascontiguousarray(np.concatenate(blocks, axis=1))

    W2p = pack_w(W2, NJ)
    W3p = pack_w(W3, NOT)
    b1t = np.ascontiguousarray(b1.reshape(NJ, P).T)
    b2t = np.ascontiguousarray(b2.reshape(NJ, P).T)

    inv = (1.0 / widths).astype(np.float32)                      # [O, R]
    biasf = ((b3[:, None] - centers) * inv).astype(np.float32)   # [O, R]
    scl = np.ascontiguousarray(
        inv.reshape(NOT, P, N_RULES).transpose(1, 0, 2).reshape(P, NOT * N_RULES))
    bia = np.ascontiguousarray(
        biasf.reshape(NOT, P, N_RULES).transpose(1, 0, 2).reshape(P, NOT * N_RULES))

    ph = params.astype(np.float16)                               # [O, R]
    dgs = np.zeros((P, NOT * N_RULES * P), np.float16)
    for ot in range(NOT):
        for r in range(N_RULES):
            idx = ot * N_RULES + r
            dgs[:, idx * P:(idx + 1) * P] = np.diag(ph[ot * P:(ot + 1) * P, r])

    eye16 = np.eye(P, dtype=np.float16)
    eyef = np.eye(P, dtype=np.float32)
    ones1 = np.ones((1, P), np.float32)

    shared = dict(W1=W1, W2p=W2p, W3p=W3p, b1t=b1t, b2t=b2t,
                  scl=scl, bia=bia, dgs=dgs, eye16=eye16, eyef=eyef, ones1=ones1)
    in_maps = []
    for i in range(N_CORES):
        xs = x[i * BL:(i + 1) * BL]                              # [BL, 32]
        xT = np.ascontiguousarray(xs.T)                          # [32, BL]
        xbp = np.ascontiguousarray(
            xs.reshape(BL // P, P, IN_DIM).transpose(1, 0, 2).reshape(P, -1))
        in_maps.append(dict(shared, xT=xT, xbp=xbp))
    return in_maps


def run(trace=False, **inputs):
    nc = _build()
    in_maps = _prepare_in_maps(**inputs)
    res = run_bass_kernel_spmd(nc, in_maps, core_ids=list(range(N_CORES)), trace=trace)
    outs = [res.results[i]["out"].T for i in range(N_CORES)]     # each [BL, O]
    full = np.ascontiguousarray(np.concatenate(outs, axis=0), dtype=np.float32)
    return full, res


def kernel(**inputs) -> np.ndarray:
    full, _ = run(trace=False, **inputs)
    return full
